# revision 11
# baseline (speedup 1.0000x reference)
"""DendriteLayer Trainium2 kernel.

Math (reference): out0 = x @ (w_in*w_in_mask).T + b_in; a = out0.reshape(B, dpc, out_dim);
winner = argmax_d(a * boost); out1 = a * one_hot(winner); y = out1f @ (w_out*dend_mask).T + b_out.

Sharding: 8 cores, core c owns global units u in [c*256, (c+1)*256) (all dpc=8 dendrites)
and output columns v with (v % 256) in [c*32, (c+1)*32). Both k-winners and the
block-diagonal output stage are then fully local to a core (no collectives).

Per-core j' layout is u'-major interleaved: j' = u'*8 + d, so the 8 dendrites of a
unit are consecutive, and each 512-wide chunk of j' is self-contained for both the
k-winners (max over d) and the output segment-sums.

Matmul: single f32r pass, out0 = Xr @ Wr with Xr/Wr the RNE-rounded f32r (12-bit
mantissa) operands, split on the HOST and DMA'd pre-tiled (the device runs only
matmuls + the k-winners/output stage). out0 rel err ~3.8e-4; the end-to-end rel
err is dominated by k-winners argmax flips between near-tied dendrites and
measures ~1.1e-2 on the fixed inputs (vs the 2e-2 tolerance) - validated on CPU
against the fp32 reference and measured on hw.

Loop structure: chunk-pairs. X batch-tiles are loaded once per half (j-chunks
{0,1} then {2,3}), halving X HBM traffic vs chunk-major order so DMA
(~90 MB/core) stays well under the ~440 us tensor-bound runtime.
"""

import numpy as np

B, IN_DIM, OUT_DIM, DPC = 4096, 2048, 2048, 8
ND = OUT_DIM * DPC
NCORES = 8
UPC = OUT_DIM // NCORES          # units per core = 256
JPC = UPC * DPC                  # j' per core = 2048
CHUNK = 512                      # j' chunk width (64 units x 8 dendrites)
NCHUNK = JPC // CHUNK            # 4
BT = 128                         # batch tile
NBT = B // BT                    # 32
KT = 128                         # k tile
NKT = IN_DIM // KT               # 16
NSTRIP = NCHUNK * NKT            # 64
YW = CHUNK // DPC                # y columns per chunk = 64
BOOST_STRENGTH = 2.0

_prog_cache = {}
LAST_RESULTS = None


def _round_f32r(a):
    """Round fp32 -> f32r (11 explicit mantissa bits), RNE. Exact bit-twiddle."""
    u = a.view(np.uint32).astype(np.uint64)
    u = u + np.uint64(0xFFF) + ((u >> np.uint64(12)) & np.uint64(1))
    u = u & np.uint64(0xFFFFF000)
    return u.astype(np.uint32).view(np.float32)


def _build(has_bin, has_bout):
    import concourse.mybir as mybir
    import concourse.tile as tile
    from concourse import bacc

    f32 = mybir.dt.float32
    f32r = mybir.dt.float32r

    nc = bacc.Bacc("TRN2", target_bir_lowering=False, debug=False)
    XR_d = nc.dram_tensor("XR", [NBT, 128, NKT * BT], f32r, kind="ExternalInput").ap()
    WR_d = nc.dram_tensor("WR", [NSTRIP, 128, CHUNK], f32r, kind="ExternalInput").ap()
    We_d = nc.dram_tensor("We", [128, JPC], f32, kind="ExternalInput").ap()
    duty_d = nc.dram_tensor("duty", [128, JPC], f32, kind="ExternalInput").ap()
    if has_bin:
        bin_d = nc.dram_tensor("bin", [128, JPC], f32, kind="ExternalInput").ap()
    if has_bout:
        bout_d = nc.dram_tensor("bout", [128, NCHUNK * YW], f32, kind="ExternalInput").ap()
    Y_d = nc.dram_tensor("Y", [NCHUNK, B, YW], f32, kind="ExternalOutput").ap()

    with tile.TileContext(nc) as tc:
        with tc.tile_pool(name="tables", bufs=1) as tbl, \
             tc.tile_pool(name="wres", bufs=2) as wres, \
             tc.tile_pool(name="xsplit", bufs=2) as xsplit, \
             tc.tile_pool(name="ypool", bufs=3) as ypool, \
             tc.tile_pool(name="st2", bufs=2) as st2, \
             tc.tile_pool(name="psum", bufs=4, space="PSUM") as psum:

            # ---- one-time tables ----
            du = tbl.tile([128, JPC], f32, name="du", tag="du")
            nc.sync.dma_start(du[:], duty_d[:])
            bo = tbl.tile([128, JPC], f32, name="bo")  # boost, broadcast on partitions
            bias_t = tbl.tile([128, 1], f32, name="bias_t")
            nc.gpsimd.memset(bias_t[:], BOOST_STRENGTH / DPC)
            scale_t = tbl.tile([128, 1], f32, name="scale_t")
            nc.gpsimd.memset(scale_t[:], -BOOST_STRENGTH)
            nc.scalar.activation(bo[:], du[:], mybir.ActivationFunctionType.Exp,
                                 bias=bias_t[:], scale=scale_t[:])
            if has_bin:
                bbt = tbl.tile([128, JPC], f32, name="bbt")
                nc.sync.dma_start(bbt[:], bin_d[:])  # plain b_in (boost applied in stage-2)
            if has_bout:
                bot = tbl.tile([128, NCHUNK * YW], f32, name="bot")
                nc.sync.dma_start(bot[:], bout_d[:])

            strips = {}

            def emit_strip(w, kt):
                wr = wres.tile([128, CHUNK], f32r, name=f"wr_{w}_{kt}",
                               tag=f"wr{w % 2}_{kt}")
                nc.scalar.dma_start(wr[:], WR_d[w * NKT + kt])
                strips[(w, kt)] = wr

            def emit_x(i):
                xr = xsplit.tile([128, NKT * BT], f32r, name=f"xr_{i}", tag="xr")
                nc.sync.dma_start(xr[:], XR_d[i])
                return xr

            xnext = None
            for half in range(2):
                w0 = 2 * half
                if half == 0:
                    # raw W_elem table for stage-2 values (before the strip
                    # burst so the first stage-2 is never blocked on it)
                    we = tbl.tile([128, JPC], f32, name="we")
                    nc.sync.dma_start(we[:], We_d[:])
                    xpre = [emit_x(0), emit_x(1)]
                    for w in (0, 1):
                        for kt in range(NKT):
                            emit_strip(w, kt)

                W2 = 2 * CHUNK
                for i in range(NBT):
                    if half == 0 and i <= 1:
                        xr = xpre[i]
                    else:
                        xr = xnext if xnext is not None else emit_x(i)

                    # both chunks of the pair accumulate into one 2-bank psum tile
                    g = psum.tile([128, W2], f32, name=f"g_{half}_{i}", tag="g")
                    for wi in range(2):
                        gsub = g[:, wi*CHUNK:(wi+1)*CHUNK]
                        for kt in range(NKT):
                            nc.tensor.matmul(gsub, xr[:, kt*BT:(kt+1)*BT],
                                             strips[(w0 + wi, kt)][:],
                                             start=(kt == 0), stop=(kt == NKT - 1))
                        if wi == 0:
                            # prefetch next b-tile's X (or the wraparound for half 1)
                            nxt = i + 1
                            if half == 0 and nxt <= 1:
                                xnext = None
                            elif nxt < NBT:
                                xnext = emit_x(nxt)
                            elif half == 0:
                                xnext = emit_x(0)
                            else:
                                xnext = None

                    # ---- stage 2 (batched over the chunk pair): k-winners +
                    # masked output segment-sum ----
                    if has_bin:
                        gs = st2.tile([128, W2], f32, name=f"gs_{half}_{i}", tag="gs")
                        nc.vector.tensor_add(gs[:], g[:], bbt[:, w0*CHUNK:(w0+2)*CHUNK])
                        gin = gs
                    else:
                        gin = g
                    gb = st2.tile([128, W2], f32, name=f"gb_{half}_{i}", tag="gb")
                    nc.vector.tensor_mul(gb[:], gin[:], bo[:, w0*CHUNK:(w0+2)*CHUNK])
                    m = st2.tile([128, W2 // DPC], f32, name=f"m_{half}_{i}", tag="m")
                    nc.vector.reduce_max(m[:], gb[:].rearrange("p (u d) -> p u d", d=DPC),
                                         axis=mybir.AxisListType.X)
                    e = st2.tile([128, W2], f32, name=f"e_{half}_{i}", tag="e")
                    mb = m[:].rearrange("p (u one) -> p u one", one=1).broadcast_to((128, W2 // DPC, DPC))
                    nc.vector.tensor_tensor(e[:].rearrange("p (u d) -> p u d", d=DPC),
                                            gb[:].rearrange("p (u d) -> p u d", d=DPC),
                                            mb, op=mybir.AluOpType.is_ge)
                    z = st2.tile([128, W2], f32, name=f"z_{half}_{i}", tag="z")
                    nc.vector.tensor_mul(z[:], gin[:], we[:, w0*CHUNK:(w0+2)*CHUNK])
                    # mask-multiply on GpSimd, scatter-written into (wi,s,q,t)
                    # order so the segment-sum below reads contiguously
                    zt = st2.tile([128, W2], f32, name=f"zt_{half}_{i}", tag="zt")
                    ztv = zt[:].rearrange("p (wi s q t) -> p wi s q t", wi=2, q=8, t=8)
                    zv = z[:].rearrange("p (wi s t q) -> p wi s q t", wi=2, t=8, q=8)
                    ev = e[:].rearrange("p (wi s t q) -> p wi s q t", wi=2, t=8, q=8)
                    nc.gpsimd.tensor_mul(ztv, zv, ev)
                    # y[p, wi*64 + 8s+q] = sum_t zt[wi,s,q,t]
                    y = ypool.tile([128, 2 * YW], f32, name=f"y_{half}_{i}", tag="y")
                    yv = y[:].rearrange("p (wi s q) -> p wi s q", wi=2, q=8)
                    nc.vector.reduce_sum(yv, zt[:].rearrange(
                        "p (wi s q t) -> p wi s q t", wi=2, q=8, t=8),
                        axis=mybir.AxisListType.X)
                    if has_bout:
                        nc.vector.tensor_add(y[:], y[:], bot[:, w0*YW:(w0+2)*YW])
                    nc.scalar.dma_start(Y_d[w0, i*BT:(i+1)*BT, :], y[:, :YW])
                    nc.scalar.dma_start(Y_d[w0 + 1, i*BT:(i+1)*BT, :], y[:, YW:])

                    # spread next half's strip DMAs over this half
                    if half == 0:
                        emit_strip(2 + i // NKT, i % NKT)

    nc.compile()
    return nc


def _tile_x(a):
    """[B, IN_DIM] -> [NBT, 128(p=k%128), NKT*BT] preserving dtype."""
    return np.ascontiguousarray(
        a.reshape(NBT, BT, NKT, 128).transpose(0, 3, 2, 1).reshape(NBT, 128, -1))


def _tile_w(a):
    """[IN_DIM, JPC] -> [NCHUNK*NKT, 128, CHUNK]."""
    return np.ascontiguousarray(
        a.reshape(NKT, 128, NCHUNK, CHUNK).transpose(2, 0, 1, 3).reshape(NSTRIP, 128, CHUNK))


def kernel(x, w_in, b_in, w_in_mask, w_out, b_out, duty_cycle):
    from concourse.bass_utils import run_bass_kernel_spmd
    global LAST_RESULTS

    x = np.ascontiguousarray(x, dtype=np.float32)
    w_in = np.asarray(w_in, dtype=np.float32)
    w_in_mask = np.asarray(w_in_mask, dtype=np.float32)
    w_out = np.asarray(w_out, dtype=np.float32)
    b_in = np.asarray(b_in, dtype=np.float32)
    b_out = np.asarray(b_out, dtype=np.float32)
    duty_cycle = np.asarray(duty_cycle, dtype=np.float32)
    assert x.shape == (B, IN_DIM) and w_in.shape == (ND, IN_DIM)

    has_bin = bool(np.any(b_in))
    has_bout = bool(np.any(b_out))

    key = (has_bin, has_bout)
    if key not in _prog_cache:
        _prog_cache[key] = _build(has_bin, has_bout)
    nc = _prog_cache[key]

    # ---- host-side operand prep: f32r rounding + device tiling ----
    XRt = _tile_x(_round_f32r(x))                         # [NBT, 128, NKT*BT] f32

    # w_in[d*OUT + c*UPC + u', k] -> per-core [k, j'=u'*8+d] via reshape/transpose
    Wmask = w_in * w_in_mask                              # [ND, IN_DIM]
    w4 = Wmask.reshape(DPC, NCORES, UPC, IN_DIM)          # [d, c, u', k]
    wof = w_out.reshape(-1)

    uprime = np.arange(UPC)
    dd = np.arange(DPC)
    jp_u = np.repeat(uprime, DPC)                         # u'(j') ; j' = u'*8 + d
    jp_d = np.tile(dd, UPC)                               # d(j')

    in_maps = []
    for c in range(NCORES):
        rows = jp_d * OUT_DIM + c * UPC + jp_u            # global w_in row per j'
        Wm = np.ascontiguousarray(w4[:, c].transpose(2, 1, 0).reshape(IN_DIM, JPC))
        WRt = _tile_w(_round_f32r(Wm))
        v = jp_d * (OUT_DIM // DPC) + c * (UPC // DPC) + (jp_u // DPC)  # d*256 + c*32 + u'//8
        t = jp_u % DPC
        We = np.broadcast_to(wof[v * ND + v * DPC + t].astype(np.float32), (128, JPC))
        duty = np.broadcast_to(duty_cycle[jp_d, c * UPC + jp_u].astype(np.float32), (128, JPC))
        im = {"XR": XRt, "WR": WRt,
              "We": np.ascontiguousarray(We), "duty": np.ascontiguousarray(duty)}
        if has_bin:
            im["bin"] = np.ascontiguousarray(np.broadcast_to(b_in[rows], (128, JPC)))
        if has_bout:
            # bout4[w*64 + s*8 + q] = b_out[v], v = q*256 + c*32 + 8w + s
            wq = np.arange(NCHUNK * YW)
            wi, si, qi = wq // YW, (wq % YW) // 8, wq % 8
            vv = qi * (OUT_DIM // DPC) + c * (UPC // DPC) + 8 * wi + si
            im["bout"] = np.ascontiguousarray(np.broadcast_to(b_out[vv], (128, NCHUNK * YW)))
        in_maps.append(im)

    import os
    trace = bool(os.environ.get("KERNEL_TRACE"))
    last_err = None
    for _attempt in range(3):
        try:
            res = run_bass_kernel_spmd(nc, in_maps, list(range(NCORES)), trace=trace)
            break
        except Exception as err:  # rare transient device fault on first execute
            last_err = err
            import time as _time
            _time.sleep(2.0)
    else:
        raise last_err
    LAST_RESULTS = res

    # Y4[w, b, s*8+q] (per core) -> y[b, q*256 + c*32 + 8w + s]
    Yc = np.stack([res.results[c]["Y"] for c in range(NCORES)], axis=0)  # [8, NCHUNK, B, 64]
    Yc = Yc.reshape(NCORES, NCHUNK, B, 8, 8)             # [c, w, b, s, q]
    y = Yc.transpose(2, 4, 0, 1, 3).reshape(B, OUT_DIM)  # [b, q, c, w, s] -> v = q*256+c*32+8w+s
    return np.ascontiguousarray(y)


# revision 14
# speedup vs baseline: 1.0663x; 1.0663x over previous
"""DendriteLayer Trainium2 kernel.

Math (reference): out0 = x @ (w_in*w_in_mask).T + b_in; a = out0.reshape(B, dpc, out_dim);
winner = argmax_d(a * boost); out1 = a * one_hot(winner); y = out1f @ (w_out*dend_mask).T + b_out.

Sharding: 8 cores, core c owns global units u in [c*256, (c+1)*256) (all dpc=8 dendrites)
and output columns v with (v % 256) in [c*32, (c+1)*32). Both k-winners and the
block-diagonal output stage are then fully local to a core (no collectives).

Per-core j' layout is u'-major interleaved: j' = u'*8 + d, so the 8 dendrites of a
unit are consecutive, and each 512-wide chunk of j' is self-contained for both the
k-winners (max over d) and the output segment-sums.

Matmul: single f32r pass, out0 = Xr @ Wr with Xr/Wr the RNE-rounded f32r (12-bit
mantissa) operands, split on the HOST and DMA'd pre-tiled (the device runs only
matmuls + the k-winners/output stage). out0 rel err ~3.8e-4; the end-to-end rel
err is dominated by k-winners argmax flips between near-tied dendrites and
measures ~1.1e-2 on the fixed inputs (vs the 2e-2 tolerance) - validated on CPU
against the fp32 reference and measured on hw.

Loop structure: chunk-pairs. X batch-tiles are loaded once per half (j-chunks
{0,1} then {2,3}), halving X HBM traffic vs chunk-major order so DMA
(~90 MB/core) stays well under the ~440 us tensor-bound runtime.
"""

import numpy as np

B, IN_DIM, OUT_DIM, DPC = 4096, 2048, 2048, 8
ND = OUT_DIM * DPC
NCORES = 8
UPC = OUT_DIM // NCORES          # units per core = 256
JPC = UPC * DPC                  # j' per core = 2048
CHUNK = 512                      # j' chunk width (64 units x 8 dendrites)
NCHUNK = JPC // CHUNK            # 4
BT = 128                         # batch tile
NBT = B // BT                    # 32
KT = 128                         # k tile
NKT = IN_DIM // KT               # 16
NSTRIP = NCHUNK * NKT            # 64
YW = CHUNK // DPC                # y columns per chunk = 64
BOOST_STRENGTH = 2.0

_prog_cache = {}
LAST_RESULTS = None


def _round_f32r(a):
    """Round fp32 -> f32r (11 explicit mantissa bits), RNE. Exact bit-twiddle."""
    u = a.view(np.uint32).astype(np.uint64)
    u = u + np.uint64(0xFFF) + ((u >> np.uint64(12)) & np.uint64(1))
    u = u & np.uint64(0xFFFFF000)
    return u.astype(np.uint32).view(np.float32)


def _build(has_bin, has_bout):
    import concourse.mybir as mybir
    import concourse.tile as tile
    from concourse import bacc

    f32 = mybir.dt.float32
    f32r = mybir.dt.float32r

    nc = bacc.Bacc("TRN2", target_bir_lowering=False, debug=False)
    XR_d = nc.dram_tensor("XR", [NBT, 128, NKT * BT], f32r, kind="ExternalInput").ap()
    WR_d = nc.dram_tensor("WR", [NSTRIP, 128, CHUNK], f32r, kind="ExternalInput").ap()
    We_d = nc.dram_tensor("We", [128, JPC], f32, kind="ExternalInput").ap()
    duty_d = nc.dram_tensor("duty", [128, JPC], f32, kind="ExternalInput").ap()
    if has_bin:
        bin_d = nc.dram_tensor("bin", [128, JPC], f32, kind="ExternalInput").ap()
    if has_bout:
        bout_d = nc.dram_tensor("bout", [128, NCHUNK * YW], f32, kind="ExternalInput").ap()
    Y_d = nc.dram_tensor("Y", [NCHUNK, B, YW], f32, kind="ExternalOutput").ap()

    with tile.TileContext(nc) as tc:
        with tc.tile_pool(name="tables", bufs=1) as tbl, \
             tc.tile_pool(name="wres", bufs=2) as wres, \
             tc.tile_pool(name="xsplit", bufs=3) as xsplit, \
             tc.tile_pool(name="ypool", bufs=3) as ypool, \
             tc.tile_pool(name="st2", bufs=2) as st2, \
             tc.tile_pool(name="psum", bufs=4, space="PSUM") as psum:

            # ---- one-time tables ----
            du = tbl.tile([128, JPC], f32, name="du", tag="du")
            nc.sync.dma_start(du[:], duty_d[:])
            bo = tbl.tile([128, JPC], f32, name="bo")  # boost, broadcast on partitions
            bias_t = tbl.tile([128, 1], f32, name="bias_t")
            nc.gpsimd.memset(bias_t[:], BOOST_STRENGTH / DPC)
            scale_t = tbl.tile([128, 1], f32, name="scale_t")
            nc.gpsimd.memset(scale_t[:], -BOOST_STRENGTH)
            nc.scalar.activation(bo[:], du[:], mybir.ActivationFunctionType.Exp,
                                 bias=bias_t[:], scale=scale_t[:])
            if has_bin:
                bbt = tbl.tile([128, JPC], f32, name="bbt")
                nc.sync.dma_start(bbt[:], bin_d[:])  # plain b_in (boost applied in stage-2)
            if has_bout:
                bot = tbl.tile([128, NCHUNK * YW], f32, name="bot")
                nc.sync.dma_start(bot[:], bout_d[:])

            strips = {}

            def emit_strip(w, kt):
                wr = wres.tile([128, CHUNK], f32r, name=f"wr_{w}_{kt}",
                               tag=f"wr{w % 2}_{kt}")
                nc.scalar.dma_start(wr[:], WR_d[w * NKT + kt])
                strips[(w, kt)] = wr

            def emit_x(i):
                xr = xsplit.tile([128, NKT * BT], f32r, name=f"xr_{i}", tag="xr")
                nc.sync.dma_start(xr[:], XR_d[i])
                return xr

            xnext = None
            for half in range(2):
                w0 = 2 * half
                if half == 0:
                    xpre = [emit_x(0), emit_x(1)]
                    for w in (0, 1):
                        for kt in range(NKT):
                            emit_strip(w, kt)
                    # raw W_elem table for stage-2 values
                    we = tbl.tile([128, JPC], f32, name="we")
                    nc.sync.dma_start(we[:], We_d[:])

                W2 = 2 * CHUNK
                for i in range(NBT):
                    if half == 0 and i <= 1:
                        xr = xpre[i]
                    else:
                        xr = xnext if xnext is not None else emit_x(i)

                    # both chunks of the pair accumulate into one 2-bank psum tile
                    g = psum.tile([128, W2], f32, name=f"g_{half}_{i}", tag="g")
                    for wi in range(2):
                        gsub = g[:, wi*CHUNK:(wi+1)*CHUNK]
                        for kt in range(NKT):
                            nc.tensor.matmul(gsub, xr[:, kt*BT:(kt+1)*BT],
                                             strips[(w0 + wi, kt)][:],
                                             start=(kt == 0), stop=(kt == NKT - 1))
                        if wi == 0:
                            # prefetch next b-tile's X (or the wraparound for half 1)
                            nxt = i + 1
                            if half == 0 and nxt <= 1:
                                xnext = None
                            elif nxt < NBT:
                                xnext = emit_x(nxt)
                            elif half == 0:
                                xnext = emit_x(0)
                            else:
                                xnext = None

                    # ---- stage 2 (batched over the chunk pair): k-winners +
                    # masked output segment-sum ----
                    if has_bin:
                        gs = st2.tile([128, W2], f32, name=f"gs_{half}_{i}", tag="gs")
                        nc.vector.tensor_add(gs[:], g[:], bbt[:, w0*CHUNK:(w0+2)*CHUNK])
                        gin = gs
                    else:
                        gin = g
                    gb = st2.tile([128, W2], f32, name=f"gb_{half}_{i}", tag="gb")
                    nc.vector.tensor_mul(gb[:], gin[:], bo[:, w0*CHUNK:(w0+2)*CHUNK])
                    m = st2.tile([128, W2 // DPC], f32, name=f"m_{half}_{i}", tag="m")
                    nc.vector.reduce_max(m[:], gb[:].rearrange("p (u d) -> p u d", d=DPC),
                                         axis=mybir.AxisListType.X)
                    e = st2.tile([128, W2], f32, name=f"e_{half}_{i}", tag="e")
                    mb = m[:].rearrange("p (u one) -> p u one", one=1).broadcast_to((128, W2 // DPC, DPC))
                    nc.vector.tensor_tensor(e[:].rearrange("p (u d) -> p u d", d=DPC),
                                            gb[:].rearrange("p (u d) -> p u d", d=DPC),
                                            mb, op=mybir.AluOpType.is_ge)
                    z = st2.tile([128, W2], f32, name=f"z_{half}_{i}", tag="z")
                    nc.vector.tensor_mul(z[:], gin[:], we[:, w0*CHUNK:(w0+2)*CHUNK])
                    nc.gpsimd.tensor_mul(z[:], z[:], e[:])
                    # y[p, wi*64 + 8s+q] = sum_t z[wi*512 + 64s + 8t + q]
                    y = ypool.tile([128, 2 * YW], f32, name=f"y_{half}_{i}", tag="y")
                    ov = z[:].rearrange("p (wi s t q) -> p wi s q t", wi=2, t=8, q=8)
                    yv = y[:].rearrange("p (wi s q) -> p wi s q", wi=2, q=8)
                    nc.vector.reduce_sum(yv, ov, axis=mybir.AxisListType.X)
                    if has_bout:
                        nc.vector.tensor_add(y[:], y[:], bot[:, w0*YW:(w0+2)*YW])
                    nc.scalar.dma_start(Y_d[w0, i*BT:(i+1)*BT, :], y[:, :YW])
                    nc.scalar.dma_start(Y_d[w0 + 1, i*BT:(i+1)*BT, :], y[:, YW:])

                    # spread next half's strip DMAs over this half
                    if half == 0:
                        emit_strip(2 + i // NKT, i % NKT)

    nc.compile()
    return nc


def _tile_x(a):
    """[B, IN_DIM] -> [NBT, 128(p=k%128), NKT*BT] preserving dtype."""
    return np.ascontiguousarray(
        a.reshape(NBT, BT, NKT, 128).transpose(0, 3, 2, 1).reshape(NBT, 128, -1))


def _tile_w(a):
    """[IN_DIM, JPC] -> [NCHUNK*NKT, 128, CHUNK]."""
    return np.ascontiguousarray(
        a.reshape(NKT, 128, NCHUNK, CHUNK).transpose(2, 0, 1, 3).reshape(NSTRIP, 128, CHUNK))


def kernel(x, w_in, b_in, w_in_mask, w_out, b_out, duty_cycle):
    from concourse.bass_utils import run_bass_kernel_spmd
    global LAST_RESULTS

    x = np.ascontiguousarray(x, dtype=np.float32)
    w_in = np.asarray(w_in, dtype=np.float32)
    w_in_mask = np.asarray(w_in_mask, dtype=np.float32)
    w_out = np.asarray(w_out, dtype=np.float32)
    b_in = np.asarray(b_in, dtype=np.float32)
    b_out = np.asarray(b_out, dtype=np.float32)
    duty_cycle = np.asarray(duty_cycle, dtype=np.float32)
    assert x.shape == (B, IN_DIM) and w_in.shape == (ND, IN_DIM)

    has_bin = bool(np.any(b_in))
    has_bout = bool(np.any(b_out))

    key = (has_bin, has_bout)
    if key not in _prog_cache:
        _prog_cache[key] = _build(has_bin, has_bout)
    nc = _prog_cache[key]

    # ---- host-side operand prep: f32r rounding + device tiling ----
    XRt = _tile_x(_round_f32r(x))                         # [NBT, 128, NKT*BT] f32

    # w_in[d*OUT + c*UPC + u', k] -> per-core [k, j'=u'*8+d] via reshape/transpose
    Wmask = w_in * w_in_mask                              # [ND, IN_DIM]
    w4 = Wmask.reshape(DPC, NCORES, UPC, IN_DIM)          # [d, c, u', k]
    wof = w_out.reshape(-1)

    uprime = np.arange(UPC)
    dd = np.arange(DPC)
    jp_u = np.repeat(uprime, DPC)                         # u'(j') ; j' = u'*8 + d
    jp_d = np.tile(dd, UPC)                               # d(j')

    in_maps = []
    for c in range(NCORES):
        rows = jp_d * OUT_DIM + c * UPC + jp_u            # global w_in row per j'
        Wm = np.ascontiguousarray(w4[:, c].transpose(2, 1, 0).reshape(IN_DIM, JPC))
        WRt = _tile_w(_round_f32r(Wm))
        v = jp_d * (OUT_DIM // DPC) + c * (UPC // DPC) + (jp_u // DPC)  # d*256 + c*32 + u'//8
        t = jp_u % DPC
        We = np.broadcast_to(wof[v * ND + v * DPC + t].astype(np.float32), (128, JPC))
        duty = np.broadcast_to(duty_cycle[jp_d, c * UPC + jp_u].astype(np.float32), (128, JPC))
        im = {"XR": XRt, "WR": WRt,
              "We": np.ascontiguousarray(We), "duty": np.ascontiguousarray(duty)}
        if has_bin:
            im["bin"] = np.ascontiguousarray(np.broadcast_to(b_in[rows], (128, JPC)))
        if has_bout:
            # bout4[w*64 + s*8 + q] = b_out[v], v = q*256 + c*32 + 8w + s
            wq = np.arange(NCHUNK * YW)
            wi, si, qi = wq // YW, (wq % YW) // 8, wq % 8
            vv = qi * (OUT_DIM // DPC) + c * (UPC // DPC) + 8 * wi + si
            im["bout"] = np.ascontiguousarray(np.broadcast_to(b_out[vv], (128, NCHUNK * YW)))
        in_maps.append(im)

    import os
    trace = bool(os.environ.get("KERNEL_TRACE"))
    last_err = None
    for _attempt in range(3):
        try:
            res = run_bass_kernel_spmd(nc, in_maps, list(range(NCORES)), trace=trace)
            break
        except Exception as err:  # rare transient device fault on first execute
            last_err = err
            import time as _time
            _time.sleep(2.0)
    else:
        raise last_err
    LAST_RESULTS = res

    # Y4[w, b, s*8+q] (per core) -> y[b, q*256 + c*32 + 8w + s]
    Yc = np.stack([res.results[c]["Y"] for c in range(NCORES)], axis=0)  # [8, NCHUNK, B, 64]
    Yc = Yc.reshape(NCORES, NCHUNK, B, 8, 8)             # [c, w, b, s, q]
    y = Yc.transpose(2, 4, 0, 1, 3).reshape(B, OUT_DIM)  # [b, q, c, w, s] -> v = q*256+c*32+8w+s
    return np.ascontiguousarray(y)
